# revision 31
# baseline (speedup 1.0000x reference)
"""
Masked-attention kernel for Trainium2, batch-sharded over 8 NeuronCores.

Model (per batch b):
    q = relu(x @ Wq + bq); k = relu(x @ Wk + bk); v = relu(x @ Wv + bv)
    S = q @ k.T ; logits = S*mask - BIG*(1-mask); att = softmax(logits, -1)
    out = att @ v

Device algorithm (transposed pipeline, no on-device transposes):
    - Host supplies xT[b] = x[b].T (bf16) and a swizzled bf16 mask^T.
    - qT = relu(Wq.T @ xT + bq) [h, i]; kT likewise [h, j]
    - v natural [j, h] via lhsT = xT j-tile, rhs = Wv; stored with a ones
      column so the softmax denominator falls out of att@v for free.
    - Pipelined over 1024-wide i-chunks (ic): for each jt, S^T tiles on
      PE -> exp on ACT ([128,1024] ops from PSUM) -> mask-mul on DVE
      (bf16 2x mode) -> P^T; the previous chunk's att@v chains are
      interleaved between score pairs so the in-order PE stream never
      starves while ACT (the bottleneck engine) drains exp.
"""

import os
import sys

sys.path.insert(0, "/opt/trn_rl_repo")

from contextlib import ExitStack

import numpy as np
import ml_dtypes

import concourse.bass as bass
import concourse.tile as tile
from concourse import bacc, mybir
from concourse.bass_utils import run_bass_kernel_spmd

N_CORES = 8
B = 16
N = 2048
D = 128
H = 128
BPC = B // N_CORES          # batches per core
NT = N // 128               # 16 j/i tiles
NIC = 2                     # i-chunks per batch
ICW = N // NIC              # i-chunk width (1024)
ITPC = ICW // 128           # i-tiles per chunk (8)
NPAIR = NT // 2             # jt pairs (8)
VW = H + 4                  # v_ext row width: h(128) + ones col + pad

BF16 = mybir.dt.bfloat16
F32 = mybir.dt.float32

_CACHE = {}


def _normalize(nc, state):
    """Normalize the oldest finished att@v group: out = num * recip(den).
    Emits the half-batch output DMA once a chunk's last i-tile is done."""
    po, osb, ic, il, out_b = state["norm"].pop(0)
    rd = state["rdp"].tile([128, 1], F32)
    nc.vector.reciprocal(out=rd[:], in_=po[:, 128:129])
    nc.vector.tensor_scalar_mul(osb[:, ic, il, :], po[:, 0:H], rd[:])
    if il == ITPC - 1:
        nc.scalar.dma_start(
            out=out_b[ic * ICW:(ic + 1) * ICW].rearrange(
                "(t p) h -> p t h", p=128),
            in_=osb[:, ic],
        )


def _attv_group(nc, state, il):
    """One att@v accumulation chain for i-tile `il` of the pending
    (previous) i-chunk. Normalization of the prior group is deferred to
    here so the in-order DVE stream never waits on a PE chain."""
    pt, vext, osb, ic, out_b = state["pending"]
    po = state["pop"].tile([128, VW], F32, tag="po")
    for jt in range(NT):
        nc.tensor.matmul(
            out=po[:],
            lhsT=pt[:, jt, il * 128:(il + 1) * 128],
            rhs=vext[:, jt, :],
            start=(jt == 0), stop=(jt == NT - 1),
        )
    state["norm"].append((po, osb, ic, il, out_b))
    if len(state["norm"]) > 1:
        _normalize(nc, state)


def _emit(nc, tc, ctx, aps):
    xt_ap = aps["xt"]          # [BPC, 128, 2048] bf16   x^T per batch
    mk_ap = aps["maskh"]       # [BPC, 8, 2, 128, 2, 1024] bf16 swizzled maskT
    wq_ap, wk_ap, wv_ap = aps["wq"], aps["wk"], aps["wv"]   # [128,128] bf16
    bq_ap, bk_ap = aps["bq"], aps["bk"]                     # [128,1] f32
    bvb_ap = aps["bvb"]        # [128, 2048] f32 (bv tiled x16, bcast parts)
    out_ap = aps["out"]        # [BPC, 2048, 128] f32

    const = ctx.enter_context(tc.tile_pool(name="const", bufs=1))
    xtp = ctx.enter_context(tc.tile_pool(name="xtp", bufs=2))
    qkp = ctx.enter_context(tc.tile_pool(name="qkp", bufs=2))
    vxp = ctx.enter_context(tc.tile_pool(name="vxp", bufs=2))
    vtp = ctx.enter_context(tc.tile_pool(name="vtp", bufs=2))
    etp = ctx.enter_context(tc.tile_pool(name="etp", bufs=6))
    mkp = ctx.enter_context(tc.tile_pool(name="mkp", bufs=6))
    ptp = ctx.enter_context(tc.tile_pool(name="ptp", bufs=2))
    osp = ctx.enter_context(tc.tile_pool(name="osp", bufs=2))
    rdp = ctx.enter_context(tc.tile_pool(name="rdp", bufs=4))
    pbig = ctx.enter_context(tc.tile_pool(name="pbig", bufs=3, space="PSUM"))
    pop = ctx.enter_context(tc.tile_pool(name="pop", bufs=2, space="PSUM"))

    # hoist the ACT exp table load off the critical path
    warm = const.tile([128, 8], F32)
    nc.vector.memset(warm[:], 0.0)
    nc.scalar.activation(out=warm[:], in_=warm[:],
                         func=mybir.ActivationFunctionType.Exp,
                         bias=0.0, scale=1.0)

    wq = const.tile([D, H], BF16)
    wk = const.tile([D, H], BF16)
    wv = const.tile([D, H], BF16)
    bq = const.tile([H, 1], F32)
    bk = const.tile([H, 1], F32)
    bvb = const.tile([128, N], BF16)
    nc.sync.dma_start(out=wq[:], in_=wq_ap[:])
    nc.sync.dma_start(out=wk[:], in_=wk_ap[:])
    nc.sync.dma_start(out=wv[:], in_=wv_ap[:])
    nc.sync.dma_start(out=bq[:], in_=bq_ap[:])
    nc.sync.dma_start(out=bk[:], in_=bk_ap[:])
    nc.scalar.dma_start(out=bvb[:], in_=bvb_ap[:])

    state = {"pending": None, "pop": pop, "rdp": rdp, "norm": []}

    def emit_proj(b):
        xt = xtp.tile([D, N], BF16)
        for s4 in range(4):
            nc.sync.dma_start(out=xt[:, s4 * 512:(s4 + 1) * 512],
                              in_=xt_ap[b, :, s4 * 512:(s4 + 1) * 512])
        qt = qkp.tile([H, N], BF16, tag="qt")
        kt = qkp.tile([H, N], BF16, tag="kt")
        for c, (dst, w, bias) in [(0, (qt, wq, bq)), (0, (kt, wk, bk)),
                                  (1, (qt, wq, bq)), (1, (kt, wk, bk))]:
            pp = pbig.tile([128, 1024], F32, tag="pp")
            for s in range(2):
                nc.tensor.matmul(
                    out=pp[:, s * 512:(s + 1) * 512],
                    lhsT=w[:],
                    rhs=xt[:, c * 1024 + s * 512: c * 1024 + (s + 1) * 512],
                    start=True, stop=True,
                )
            if dst is kt:
                # k relu on DVE: rebalance work off the ACT bottleneck
                nc.vector.tensor_scalar(
                    out=dst[:, c * 1024:(c + 1) * 1024], in0=pp[:],
                    scalar1=bias[:], scalar2=0.0,
                    op0=mybir.AluOpType.add, op1=mybir.AluOpType.max,
                )
            else:
                nc.scalar.activation(
                    out=dst[:, c * 1024:(c + 1) * 1024], in_=pp[:],
                    func=mybir.ActivationFunctionType.Relu,
                    bias=bias[:], scale=1.0,
                )
        return xt, qt, kt

    def emit_vext(xt):
        # v_ext: [j(128), jt(16), VW] ; [:, :, 0:128]=v, [:, :, 128]=1
        vext = vxp.tile([128, NT, VW], BF16)
        nc.vector.memset(vext[:], 0.0)
        nc.vector.memset(vext[:, :, 128:129], 1.0)
        for c in range(2):
            pv = pbig.tile([128, 1024], F32, tag="pp")
            for t in range(8):
                jt = 8 * c + t
                nc.tensor.matmul(
                    out=pv[:, t * 128:(t + 1) * 128],
                    lhsT=xt[:, jt * 128:(jt + 1) * 128],
                    rhs=wv[:],
                    start=True, stop=True,
                )
            vtmp = vtp.tile([128, 1024], BF16)
            nc.vector.tensor_add(vtmp[:], pv[:], bvb[:, c * 1024:(c + 1) * 1024])
            nc.vector.tensor_scalar_max(
                vext[:, 8 * c:8 * (c + 1), 0:128],
                vtmp[:].rearrange("p (a h) -> p a h", a=8),
                0.0,
            )
        return vext

    handles = emit_proj(0)
    for b in range(BPC):
        xt, qt, kt = handles
        vext = None
        osb = osp.tile([128, NIC, ITPC, H], F32)
        for ic in range(NIC):
            # scores^T -> exp -> mask-mul -> P^T for this i-chunk, with the
            # previous chunk's att@v interleaved so PE never starves, and
            # the next batch's projections injected late in the last chunk
            pt = ptp.tile([128, NT, ICW], BF16)
            for pr in range(NPAIR):
                if state["pending"] is not None:
                    _attv_group(nc, state, pr)
                if b + 1 < BPC and ic == NIC - 1 and pr == 4:
                    handles = emit_proj(b + 1)
                mk = mkp.tile([128, 2, ICW], BF16)
                nc.sync.dma_start(out=mk[:], in_=mk_ap[b, pr, ic])
                for u in range(2):
                    jt = 2 * pr + u
                    ps = pbig.tile([128, 1024], F32, tag="pp")
                    for s in range(2):
                        nc.tensor.matmul(
                            out=ps[:, s * 512:(s + 1) * 512],
                            lhsT=kt[:, jt * 128:(jt + 1) * 128],
                            rhs=qt[:, ic * ICW + s * 512: ic * ICW + (s + 1) * 512],
                            start=True, stop=True,
                        )
                    et = etp.tile([128, 1024], BF16)
                    nc.scalar.activation(
                        out=et[:], in_=ps[:],
                        func=mybir.ActivationFunctionType.Exp,
                        bias=0.0, scale=1.0,
                    )
                    nc.vector.tensor_mul(pt[:, jt, :], et[:], mk[:, u, :])
            if vext is None:
                vext = emit_vext(xt)
            state["pending"] = (pt, vext, osb, ic, out_ap[b])

    # drain the last chunk's att@v
    for il in range(ITPC):
        _attv_group(nc, state, il)
    while state["norm"]:
        _normalize(nc, state)


def _build():
    if "nc" in _CACHE:
        return _CACHE["nc"]
    nc = bacc.Bacc("TRN2", target_bir_lowering=False, debug=False,
                   num_devices=N_CORES)
    aps = {
        "xt": nc.dram_tensor("xt", [BPC, D, N], BF16, kind="ExternalInput").ap(),
        "maskh": nc.dram_tensor("maskh", [BPC, NPAIR, NIC, 128, 2, ICW], BF16,
                                kind="ExternalInput").ap(),
        "wq": nc.dram_tensor("wq", [D, H], BF16, kind="ExternalInput").ap(),
        "wk": nc.dram_tensor("wk", [D, H], BF16, kind="ExternalInput").ap(),
        "wv": nc.dram_tensor("wv", [D, H], BF16, kind="ExternalInput").ap(),
        "bq": nc.dram_tensor("bq", [H, 1], F32, kind="ExternalInput").ap(),
        "bk": nc.dram_tensor("bk", [H, 1], F32, kind="ExternalInput").ap(),
        "bvb": nc.dram_tensor("bvb", [128, N], BF16, kind="ExternalInput").ap(),
        "out": nc.dram_tensor("out", [BPC, N, H], F32, kind="ExternalOutput").ap(),
    }
    with tile.TileContext(nc) as tc, ExitStack() as ctx:
        _emit(nc, tc, ctx, aps)
    nc.compile()
    _CACHE["nc"] = nc
    return nc


def _prep_mask(mask):
    """mask [B, i, j] f32 -> swizzled bf16 H[b, pr, ic, p, u, i1] =
    mask[b, ic*1024+i1, (2*pr+u)*128+p]."""
    bf16 = ml_dtypes.bfloat16
    v = mask.reshape(B, NIC, ICW, NPAIR, 2, 128)      # [b, ic, i1, pr, u, p]
    return np.ascontiguousarray(
        v.transpose(0, 3, 1, 5, 4, 2)).astype(bf16)   # [b, pr, ic, p, u, i1]


def kernel(x, mask, Wv, bv, Wk, bk, Wq, bq):
    bf16 = ml_dtypes.bfloat16
    x = np.asarray(x, dtype=np.float32)
    mask = np.asarray(mask, dtype=np.float32)
    Wv, bv = np.asarray(Wv, np.float32), np.asarray(bv, np.float32)
    Wk, bk = np.asarray(Wk, np.float32), np.asarray(bk, np.float32)
    Wq, bq = np.asarray(Wq, np.float32), np.asarray(bq, np.float32)

    xt = x.transpose(0, 2, 1).astype(bf16)          # [B, 128, 2048]
    maskh = _prep_mask(mask)
    wq_b, wk_b, wv_b = Wq.astype(bf16), Wk.astype(bf16), Wv.astype(bf16)
    bq_c = bq.reshape(H, 1).astype(np.float32)
    bk_c = bk.reshape(H, 1).astype(np.float32)
    bvb = np.ascontiguousarray(
        np.broadcast_to(np.tile(bv, NT), (128, N))).astype(bf16)

    nc = _build()
    in_maps = []
    for c in range(N_CORES):
        sl = slice(c * BPC, (c + 1) * BPC)
        in_maps.append({
            "xt": xt[sl], "maskh": maskh[sl],
            "wq": wq_b, "wk": wk_b, "wv": wv_b,
            "bq": bq_c, "bk": bk_c, "bvb": bvb,
        })

    trace = bool(int(os.environ.get("KERNEL_TRACE", "0")))
    res = run_bass_kernel_spmd(nc, in_maps, core_ids=list(range(N_CORES)),
                               trace=trace)
    if res.exec_time_ns is not None:
        print(f"HW exec time: {res.exec_time_ns} ns")
    _CACHE["last_result"] = res
    out = np.concatenate([res.results[c]["out"] for c in range(N_CORES)], axis=0)
    return out.astype(np.float32)


# revision 36
# speedup vs baseline: 1.2360x; 1.2360x over previous
"""
Masked-attention kernel for Trainium2, batch-sharded over 8 NeuronCores.

Model (per batch b):
    q = relu(x @ Wq + bq); k = relu(x @ Wk + bk); v = relu(x @ Wv + bv)
    S = q @ k.T ; logits = S*mask - BIG*(1-mask); att = softmax(logits, -1)
    out = att @ v

Device algorithm (transposed pipeline, no on-device transposes):
    - Host supplies xT[b] = x[b].T (bf16) and a swizzled bf16 mask^T.
    - qT = relu(Wq.T @ xT + bq) [h, i]; kT likewise [h, j]
    - v natural [j, h] via lhsT = xT j-tile, rhs = Wv; stored with a ones
      column so the softmax denominator falls out of att@v for free.
    - Pipelined over 1024-wide i-chunks (ic): for each jt, S^T tiles on
      PE -> exp on ACT ([128,1024] ops from PSUM) -> mask-mul on DVE
      (bf16 2x mode) -> P^T; the previous chunk's att@v chains are
      interleaved between score pairs so the in-order PE stream never
      starves while ACT (the bottleneck engine) drains exp.
"""

import os
import sys

sys.path.insert(0, "/opt/trn_rl_repo")

from contextlib import ExitStack

import numpy as np
import ml_dtypes

import concourse.bass as bass
import concourse.tile as tile
from concourse import bacc, mybir
from concourse.bass_utils import run_bass_kernel_spmd

N_CORES = 8
B = 16
N = 2048
D = 128
H = 128
BPC = B // N_CORES          # batches per core
NT = N // 128               # 16 j/i tiles
NIC = 2                     # i-chunks per batch
ICW = N // NIC              # i-chunk width (1024)
ITPC = ICW // 128           # i-tiles per chunk (8)
NPAIR = NT // 2             # jt pairs (8)
VW = H + 4                  # v_ext row width: h(128) + ones col + pad

BF16 = mybir.dt.bfloat16
F32 = mybir.dt.float32

_CACHE = {}


def _normalize(nc, state):
    """Normalize the oldest finished att@v group: out = num * recip(den).
    Emits the half-batch output DMA once a chunk's last i-tile is done."""
    po, osb, ic, il, out_b = state["norm"].pop(0)
    rd = state["rdp"].tile([128, 1], F32)
    nc.vector.reciprocal(out=rd[:], in_=po[:, 128:129])
    nc.vector.tensor_scalar_mul(osb[:, ic, il, :], po[:, 0:H], rd[:])
    if il % 2 == 1:
        # stream out every two finished i-tiles so the drain tail only
        # waits on a 128KB residual store, not the whole half-batch
        lo = ic * ICW + (il - 1) * 128
        nc.scalar.dma_start(
            out=out_b[lo:lo + 256].rearrange("(t p) h -> p t h", p=128),
            in_=osb[:, ic, il - 1:il + 1],
        )


def _attv_group(nc, state, il):
    """One att@v accumulation chain for i-tile `il` of the pending
    (previous) i-chunk. Normalization of the prior group is deferred to
    here so the in-order DVE stream never waits on a PE chain."""
    pt, vext, osb, ic, out_b = state["pending"]
    po = state["pop"].tile([128, VW], F32, tag="po")
    for jt in range(NT):
        nc.tensor.matmul(
            out=po[:],
            lhsT=pt[:, jt, il * 128:(il + 1) * 128],
            rhs=vext[:, jt, :],
            start=(jt == 0), stop=(jt == NT - 1),
        )
    state["norm"].append((po, osb, ic, il, out_b))
    if len(state["norm"]) > 1:
        _normalize(nc, state)


def _emit(nc, tc, ctx, aps):
    xt_ap = aps["xt"]          # [BPC, 128, 2048] bf16   x^T per batch
    mk_ap = aps["maskh"]       # [BPC, 8, 2, 128, 2, 1024] bf16 swizzled maskT
    wq_ap, wk_ap, wv_ap = aps["wq"], aps["wk"], aps["wv"]   # [128,128] bf16
    bq_ap, bk_ap = aps["bq"], aps["bk"]                     # [128,1] f32
    bvb_ap = aps["bvb"]        # [128, 2048] f32 (bv tiled x16, bcast parts)
    out_ap = aps["out"]        # [BPC, 2048, 128] f32

    const = ctx.enter_context(tc.tile_pool(name="const", bufs=1))
    xtp = ctx.enter_context(tc.tile_pool(name="xtp", bufs=2))
    qkp = ctx.enter_context(tc.tile_pool(name="qkp", bufs=2))
    vxp = ctx.enter_context(tc.tile_pool(name="vxp", bufs=2))
    vtp = ctx.enter_context(tc.tile_pool(name="vtp", bufs=2))
    etp = ctx.enter_context(tc.tile_pool(name="etp", bufs=6))
    mkp = ctx.enter_context(tc.tile_pool(name="mkp", bufs=6))
    ptp = ctx.enter_context(tc.tile_pool(name="ptp", bufs=2))
    osp = ctx.enter_context(tc.tile_pool(name="osp", bufs=2))
    rdp = ctx.enter_context(tc.tile_pool(name="rdp", bufs=4))
    pbig = ctx.enter_context(tc.tile_pool(name="pbig", bufs=3, space="PSUM"))
    pop = ctx.enter_context(tc.tile_pool(name="pop", bufs=2, space="PSUM"))

    # hoist the ACT exp table load off the critical path
    warm = const.tile([128, 8], F32)
    nc.vector.memset(warm[:], 0.0)
    nc.scalar.activation(out=warm[:], in_=warm[:],
                         func=mybir.ActivationFunctionType.Exp,
                         bias=0.0, scale=1.0)

    wq = const.tile([D, H], BF16)
    wk = const.tile([D, H], BF16)
    wv = const.tile([D, H], BF16)
    bq = const.tile([H, 1], F32)
    bk = const.tile([H, 1], F32)
    bvb = const.tile([128, N], BF16)
    nc.sync.dma_start(out=wq[:], in_=wq_ap[:])
    nc.sync.dma_start(out=wk[:], in_=wk_ap[:])
    nc.sync.dma_start(out=wv[:], in_=wv_ap[:])
    nc.sync.dma_start(out=bq[:], in_=bq_ap[:])
    nc.sync.dma_start(out=bk[:], in_=bk_ap[:])
    nc.scalar.dma_start(out=bvb[:], in_=bvb_ap[:])

    state = {"pending": None, "pop": pop, "rdp": rdp, "norm": []}

    def emit_proj(b):
        xt = xtp.tile([D, N], BF16)
        for s4 in range(4):
            nc.sync.dma_start(out=xt[:, s4 * 512:(s4 + 1) * 512],
                              in_=xt_ap[b, :, s4 * 512:(s4 + 1) * 512])
        qt = qkp.tile([H, N], BF16, tag="qt")
        kt = qkp.tile([H, N], BF16, tag="kt")
        for c, (dst, w, bias) in [(0, (qt, wq, bq)), (0, (kt, wk, bk)),
                                  (1, (qt, wq, bq)), (1, (kt, wk, bk))]:
            pp = pbig.tile([128, 1024], F32, tag="pp")
            for s in range(2):
                nc.tensor.matmul(
                    out=pp[:, s * 512:(s + 1) * 512],
                    lhsT=w[:],
                    rhs=xt[:, c * 1024 + s * 512: c * 1024 + (s + 1) * 512],
                    start=True, stop=True,
                )
            if dst is kt:
                # k relu on DVE: rebalance work off the ACT bottleneck
                nc.vector.tensor_scalar(
                    out=dst[:, c * 1024:(c + 1) * 1024], in0=pp[:],
                    scalar1=bias[:], scalar2=0.0,
                    op0=mybir.AluOpType.add, op1=mybir.AluOpType.max,
                )
            else:
                nc.scalar.activation(
                    out=dst[:, c * 1024:(c + 1) * 1024], in_=pp[:],
                    func=mybir.ActivationFunctionType.Relu,
                    bias=bias[:], scale=1.0,
                )
        return xt, qt, kt

    def emit_vext(xt):
        # v_ext: [j(128), jt(16), VW] ; [:, :, 0:128]=v, [:, :, 128]=1
        vext = vxp.tile([128, NT, VW], BF16)
        nc.vector.memset(vext[:], 0.0)
        nc.vector.memset(vext[:, :, 128:129], 1.0)
        for c in range(2):
            pv = pbig.tile([128, 1024], F32, tag="pp")
            for t in range(8):
                jt = 8 * c + t
                nc.tensor.matmul(
                    out=pv[:, t * 128:(t + 1) * 128],
                    lhsT=xt[:, jt * 128:(jt + 1) * 128],
                    rhs=wv[:],
                    start=True, stop=True,
                )
            vtmp = vtp.tile([128, 1024], BF16)
            nc.vector.tensor_add(vtmp[:], pv[:], bvb[:, c * 1024:(c + 1) * 1024])
            nc.vector.tensor_scalar_max(
                vext[:, 8 * c:8 * (c + 1), 0:128],
                vtmp[:].rearrange("p (a h) -> p a h", a=8),
                0.0,
            )
        return vext

    def emit_proj_tiles(b):
        xt = xtp.tile([D, N], BF16)
        for s4 in range(4):
            nc.sync.dma_start(out=xt[:, s4 * 512:(s4 + 1) * 512],
                              in_=xt_ap[b, :, s4 * 512:(s4 + 1) * 512])
        qt = qkp.tile([H, N], BF16, tag="qt")
        kt = qkp.tile([H, N], BF16, tag="kt")
        return xt, qt, kt

    def emit_proj_chunk(h, idx):
        xt, qt, kt = h
        c, dst, w, bias = [(0, qt, wq, bq), (0, kt, wk, bk),
                           (1, qt, wq, bq), (1, kt, wk, bk)][idx]
        pp = pbig.tile([128, 1024], F32, tag="pp")
        for s in range(2):
            nc.tensor.matmul(
                out=pp[:, s * 512:(s + 1) * 512],
                lhsT=w[:],
                rhs=xt[:, c * 1024 + s * 512: c * 1024 + (s + 1) * 512],
                start=True, stop=True,
            )
        nc.vector.tensor_scalar(
            out=dst[:, c * 1024:(c + 1) * 1024], in0=pp[:],
            scalar1=bias[:], scalar2=0.0,
            op0=mybir.AluOpType.add, op1=mybir.AluOpType.max,
        )

    handles = emit_proj(0)
    for b in range(BPC):
        xt, qt, kt = handles
        vext = None
        osb = osp.tile([128, NIC, ITPC, H], F32)
        for ic in range(NIC):
            # scores^T -> exp -> mask-mul -> P^T for this i-chunk, with the
            # previous chunk's att@v interleaved so PE never starves, and
            # the next batch's projections injected late in the last chunk
            pt = ptp.tile([128, NT, ICW], BF16)
            for pr in range(NPAIR):
                if state["pending"] is not None:
                    _attv_group(nc, state, pr)
                if b + 1 < BPC and ic == 0 and 2 <= pr <= 5:
                    if pr == 2:
                        state["next"] = emit_proj_tiles(b + 1)
                    emit_proj_chunk(state["next"], pr - 2)
                    if pr == 5:
                        handles = state["next"]
                mk = mkp.tile([128, 2, ICW], BF16)
                nc.sync.dma_start(out=mk[:], in_=mk_ap[b, pr, ic])
                for u in range(2):
                    jt = 2 * pr + u
                    ps = pbig.tile([128, 1024], F32, tag="pp")
                    for s in range(2):
                        nc.tensor.matmul(
                            out=ps[:, s * 512:(s + 1) * 512],
                            lhsT=kt[:, jt * 128:(jt + 1) * 128],
                            rhs=qt[:, ic * ICW + s * 512: ic * ICW + (s + 1) * 512],
                            start=True, stop=True,
                        )
                    et = etp.tile([128, 1024], BF16)
                    nc.scalar.activation(
                        out=et[:], in_=ps[:],
                        func=mybir.ActivationFunctionType.Exp,
                        bias=0.0, scale=1.0,
                    )
                    nc.vector.tensor_mul(pt[:, jt, :], et[:], mk[:, u, :])
            if vext is None:
                vext = emit_vext(xt)
            state["pending"] = (pt, vext, osb, ic, out_ap[b])

    # drain the last chunk's att@v
    for il in range(ITPC):
        _attv_group(nc, state, il)
    while state["norm"]:
        _normalize(nc, state)


def _build():
    if "nc" in _CACHE:
        return _CACHE["nc"]
    nc = bacc.Bacc("TRN2", target_bir_lowering=False, debug=False,
                   num_devices=N_CORES)
    aps = {
        "xt": nc.dram_tensor("xt", [BPC, D, N], BF16, kind="ExternalInput").ap(),
        "maskh": nc.dram_tensor("maskh", [BPC, NPAIR, NIC, 128, 2, ICW], BF16,
                                kind="ExternalInput").ap(),
        "wq": nc.dram_tensor("wq", [D, H], BF16, kind="ExternalInput").ap(),
        "wk": nc.dram_tensor("wk", [D, H], BF16, kind="ExternalInput").ap(),
        "wv": nc.dram_tensor("wv", [D, H], BF16, kind="ExternalInput").ap(),
        "bq": nc.dram_tensor("bq", [H, 1], F32, kind="ExternalInput").ap(),
        "bk": nc.dram_tensor("bk", [H, 1], F32, kind="ExternalInput").ap(),
        "bvb": nc.dram_tensor("bvb", [128, N], BF16, kind="ExternalInput").ap(),
        "out": nc.dram_tensor("out", [BPC, N, H], F32, kind="ExternalOutput").ap(),
    }
    with tile.TileContext(nc) as tc, ExitStack() as ctx:
        _emit(nc, tc, ctx, aps)
    nc.compile()
    _CACHE["nc"] = nc
    return nc


def _prep_mask(mask):
    """mask [B, i, j] f32 -> swizzled bf16 H[b, pr, ic, p, u, i1] =
    mask[b, ic*1024+i1, (2*pr+u)*128+p]."""
    bf16 = ml_dtypes.bfloat16
    v = mask.reshape(B, NIC, ICW, NPAIR, 2, 128)      # [b, ic, i1, pr, u, p]
    return np.ascontiguousarray(
        v.transpose(0, 3, 1, 5, 4, 2)).astype(bf16)   # [b, pr, ic, p, u, i1]


def kernel(x, mask, Wv, bv, Wk, bk, Wq, bq):
    bf16 = ml_dtypes.bfloat16
    x = np.asarray(x, dtype=np.float32)
    mask = np.asarray(mask, dtype=np.float32)
    Wv, bv = np.asarray(Wv, np.float32), np.asarray(bv, np.float32)
    Wk, bk = np.asarray(Wk, np.float32), np.asarray(bk, np.float32)
    Wq, bq = np.asarray(Wq, np.float32), np.asarray(bq, np.float32)

    xt = x.transpose(0, 2, 1).astype(bf16)          # [B, 128, 2048]
    maskh = _prep_mask(mask)
    wq_b, wk_b, wv_b = Wq.astype(bf16), Wk.astype(bf16), Wv.astype(bf16)
    bq_c = bq.reshape(H, 1).astype(np.float32)
    bk_c = bk.reshape(H, 1).astype(np.float32)
    bvb = np.ascontiguousarray(
        np.broadcast_to(np.tile(bv, NT), (128, N))).astype(bf16)

    nc = _build()
    in_maps = []
    for c in range(N_CORES):
        sl = slice(c * BPC, (c + 1) * BPC)
        in_maps.append({
            "xt": xt[sl], "maskh": maskh[sl],
            "wq": wq_b, "wk": wk_b, "wv": wv_b,
            "bq": bq_c, "bk": bk_c, "bvb": bvb,
        })

    trace = bool(int(os.environ.get("KERNEL_TRACE", "0")))
    res = run_bass_kernel_spmd(nc, in_maps, core_ids=list(range(N_CORES)),
                               trace=trace)
    if res.exec_time_ns is not None:
        print(f"HW exec time: {res.exec_time_ns} ns")
    _CACHE["last_result"] = res
    out = np.concatenate([res.results[c]["out"] for c in range(N_CORES)], axis=0)
    return out.astype(np.float32)
